# revision 4
# baseline (speedup 1.0000x reference)
"""BitNet attention (B=2, S=2048, H=4096, 32 heads x 128) on 8 Trainium2 cores.

Tensor-parallel over heads: each core owns 4 heads (512 of 4096 hidden
channels). Per core:
  phase A: qT/kT (d-major) and v (token-major) projections, bf16 matmuls
           with fp32 PSUM accumulation; spilled to DRAM scratch.
  phase B: per (batch, head): RoPE on reload (normal + partition-swapped
           slices), scoresT = kT.T @ qT per (k-tile, q-block), exp on ACT,
           causal handled by skipping fully-masked k-tiles and zeroing
           diagonal tiles post-exp with gpsimd affine_select; softmax sums
           via a bf16 pairwise tree + an all-ones [128,128] matmul (yields
           the partition-sum broadcast across partitions); attn_outT
           accumulated in PSUM and evicted with the reciprocal normalizer.
  phase C: o_proj partial (contraction over the core's 512 channels),
           scaled by `so`, written token-major.
Host sums the 8 partials.
"""
import math
import sys

import numpy as np
import ml_dtypes

for _p in ("/opt/trn_rl_repo", "/opt/pypackages"):
    if _p not in sys.path:
        sys.path.append(_p)

import concourse.bass as bass
import concourse.mybir as mybir
import concourse.tile as tile
from concourse.bass_utils import run_bass_kernel_spmd

bf16 = ml_dtypes.bfloat16
F32 = mybir.dt.float32
BF16 = mybir.dt.bfloat16

B, S, H = 2, 2048, 4096
NH, D = 32, 128
NCORES = 8
HPC = NH // NCORES          # heads per core = 4
OC = HPC * D                # per-core projection width = 512
T = B * S                   # tokens = 4096
TB = 512                    # token block in phase A
NTB = T // TB               # 8
NKT = S // 128              # 16 k-tiles per sequence
NQB = S // 512              # 4 q-blocks per sequence
INV_SQRT_D = 1.0 / math.sqrt(D)

_prog_cache = {}


# ---------------------------------------------------------------------------
# BIR post-pass: this walrus build allows only ONE embedded sync-wait per
# instruction; Tile attaches several. Hoist the excess onto standalone
# EventSemaphore instructions right before the instruction (same engine
# queue, identical semantics: waits execute at the issuing sequencer).
# ---------------------------------------------------------------------------
def _split_waits(nc):
    import orjson
    import bass_rust

    m = orjson.loads(nc.to_json_str())
    counter = [0]

    def fix_block(block):
        insts = block.get("instructions")
        if not insts:
            return
        new_insts = []
        for inst in insts:
            if inst.get("opcode") != "EventSemaphore":
                si = inst.get("sync_info") or {}
                waits = si.get("on_wait") or []
                if len(waits) > 1:
                    for w in waits[:-1]:
                        counter[0] += 1
                        new_insts.append({
                            "name": f"I-{900000 + counter[0]}",
                            "opcode": "EventSemaphore",
                            "engine": inst.get("engine", "Unassigned"),
                            "debug": inst.get("debug", 0),
                            "ins": [],
                            "outs": [],
                            "sync_info": {"on_update": [], "on_wait": [w]},
                        })
                    si["on_wait"] = [waits[-1]]
            new_insts.append(inst)
        block["instructions"] = new_insts

    def walk(obj):
        if isinstance(obj, dict):
            if isinstance(obj.get("instructions"), list):
                fix_block(obj)
                for i in obj["instructions"]:
                    walk(i)
            else:
                for v in obj.values():
                    walk(v)
        elif isinstance(obj, list):
            for v in obj:
                walk(v)

    walk(m)
    nc.m = bass_rust.module_from_json_bytes(orjson.dumps(m))


def _rope_tables():
    inv_freq = 1.0 / (10000.0 ** (np.arange(0, D, 2, dtype=np.float32) / D))
    t = np.arange(S, dtype=np.float32)
    freqs = np.outer(t, inv_freq)
    emb = np.concatenate([freqs, freqs], axis=-1)        # [S, D]
    cosT = np.cos(emb).T.astype(np.float32)              # [D, S]
    sinT = np.sin(emb).T.astype(np.float32)
    sinT[: D // 2] *= -1.0                               # sign of rotate_half
    return cosT.astype(bf16), sinT.astype(bf16)


def _build(mask_mode, sq, sk, sv, so):
    """mask_mode: 'causal' | 'zeros' | 'general'."""
    nc = bass.Bass()

    xT_p = nc.declare_dram_parameter("xT", [H, T], BF16, isOutput=False)
    wqT_p = nc.declare_dram_parameter("wqT", [H, OC], BF16, isOutput=False)
    wkT_p = nc.declare_dram_parameter("wkT", [H, OC], BF16, isOutput=False)
    wvT_p = nc.declare_dram_parameter("wvT", [H, OC], BF16, isOutput=False)
    woT_p = nc.declare_dram_parameter("woT", [OC, H], BF16, isOutput=False)
    if mask_mode == "general":
        maskT_p = nc.declare_dram_parameter("maskT", [B, S, S], F32, isOutput=False)
    out_p = nc.declare_dram_parameter("out", [T, H], F32, isOutput=True)

    cos_np, sin_np = _rope_tables()
    cos_d = nc.inline_tensor(np.ascontiguousarray(cos_np), name="cos_tab")
    sin_d = nc.inline_tensor(np.ascontiguousarray(sin_np), name="sin_tab")
    ones_d = nc.inline_tensor(np.ones((128, 128), dtype=bf16), name="ones_sq")

    q_sp = nc.dram_tensor("q_spill", [OC, T], BF16)
    k_sp = nc.dram_tensor("k_spill", [OC, T], BF16)
    v_sp = nc.dram_tensor("v_spill", [T, OC], BF16)

    with tile.TileContext(nc) as tc:
        # ---------------- phase A: projections ----------------
        with (
            tc.tile_pool(name="wpool", bufs=1) as wpool,
            tc.tile_pool(name="xpool", bufs=3) as xpool,
            tc.tile_pool(name="evq", bufs=4) as evq,
            tc.tile_pool(name="pjps", bufs=4, space="PSUM") as pjps,
        ):
            wq_sb = wpool.tile([128, 32, OC], BF16, tag="wq")
            wk_sb = wpool.tile([128, 32, OC], BF16, tag="wk")
            wv_sb = wpool.tile([128, 32, OC], BF16, tag="wv")
            nc.sync.dma_start(out=wq_sb[:], in_=wqT_p[:, :].rearrange("(kt p) o -> p kt o", p=128))
            nc.sync.dma_start(out=wk_sb[:], in_=wkT_p[:, :].rearrange("(kt p) o -> p kt o", p=128))
            nc.sync.dma_start(out=wv_sb[:], in_=wvT_p[:, :].rearrange("(kt p) o -> p kt o", p=128))

            for tb in range(NTB):
                xh = []
                for half in range(2):
                    xt = xpool.tile([128, 16, TB], BF16, tag="xh")
                    nc.sync.dma_start(
                        out=xt[:],
                        in_=xT_p[half * 2048:(half + 1) * 2048, tb * TB:(tb + 1) * TB]
                        .rearrange("(kt p) t -> p kt t", p=128),
                    )
                    xh.append(xt)

                # q and k: out[o-tile, t] accumulated over 32 h-tiles
                for (w_sb, spill, s_imm, tag) in ((wq_sb, q_sp, sq, "q"), (wk_sb, k_sp, sk, "k")):
                    for ot in range(OC // 128):
                        ps = pjps.tile([128, TB], F32, tag="pj")
                        for kt in range(32):
                            nc.tensor.matmul(
                                ps[:],
                                w_sb[:, kt, ot * 128:(ot + 1) * 128],
                                xh[kt // 16][:, kt % 16, :],
                                start=(kt == 0),
                                stop=(kt == 31),
                            )
                        ev = evq.tile([128, TB], BF16, tag="ev")
                        nc.scalar.activation(ev[:], ps[:], mybir.ActivationFunctionType.Copy, scale=float(s_imm))
                        nc.sync.dma_start(
                            out=spill[ot * 128:(ot + 1) * 128, tb * TB:(tb + 1) * TB],
                            in_=ev[:],
                        )

                # v: out[t-tile, o] accumulated over 32 h-tiles
                for tt in range(TB // 128):
                    ps = pjps.tile([128, OC], F32, tag="pj")
                    for kt in range(32):
                        nc.tensor.matmul(
                            ps[:],
                            xh[kt // 16][:, kt % 16, tt * 128:(tt + 1) * 128],
                            wv_sb[:, kt, :],
                            start=(kt == 0),
                            stop=(kt == 31),
                        )
                    ev = evq.tile([128, OC], BF16, tag="ev")
                    nc.scalar.activation(ev[:], ps[:], mybir.ActivationFunctionType.Copy, scale=float(sv))
                    nc.sync.dma_start(
                        out=v_sp[tb * TB + tt * 128: tb * TB + (tt + 1) * 128, :],
                        in_=ev[:],
                    )

        # ---------------- phases B + C ----------------
        with (
            tc.tile_pool(name="consts", bufs=1) as consts,
            tc.tile_pool(name="ao", bufs=1) as ao,
        ):
            cos_sb = consts.tile([128, S], BF16, tag="cos")
            sin_sb = consts.tile([128, S], BF16, tag="sin")
            ones_sb = consts.tile([128, 128], BF16, tag="ones")
            nc.sync.dma_start(out=cos_sb[:], in_=cos_d[:])
            nc.sync.dma_start(out=sin_sb[:], in_=sin_d[:])
            nc.sync.dma_start(out=ones_sb[:], in_=ones_d[:])

            ao_sb = ao.tile([128, HPC, T], BF16, tag="ao")

            phase_b = (
                tc.tile_pool(name="hload", bufs=2),
                tc.tile_pool(name="et", bufs=22),
                tc.tile_pool(name="etd", bufs=6),
                tc.tile_pool(name="tr", bufs=18),
                tc.tile_pool(name="rcp", bufs=3),
                tc.tile_pool(name="mload", bufs=3),
                tc.tile_pool(name="ps_s", bufs=3, space="PSUM"),
                tc.tile_pool(name="ps_sum", bufs=2, space="PSUM"),
                tc.tile_pool(name="ps_o", bufs=2, space="PSUM"),
            )
            hload, et, etd, tr, rcp, mload, ps_s, ps_sum, ps_o = [
                p.__enter__() for p in phase_b]

            for b in range(B):
                for h in range(HPC):
                    row0 = h * 128
                    csl = slice(b * S, (b + 1) * S)

                    def load_rot(spill, tagp):
                        raw = hload.tile([128, S], BF16, tag=tagp + "raw")
                        nc.sync.dma_start(out=raw[:], in_=spill[row0:row0 + 128, csl])
                        sw = hload.tile([128, S], BF16, tag=tagp + "sw")
                        nc.sync.dma_start(out=sw[0:64, :], in_=spill[row0 + 64:row0 + 128, csl])
                        nc.sync.dma_start(out=sw[64:128, :], in_=spill[row0:row0 + 64, csl])
                        nc.vector.tensor_mul(raw[:], raw[:], cos_sb[:])
                        nc.vector.tensor_mul(sw[:], sw[:], sin_sb[:])
                        nc.vector.tensor_add(raw[:], raw[:], sw[:])
                        return raw

                    qr = load_rot(q_sp, "q")
                    kr = load_rot(k_sp, "k")

                    v_h = hload.tile([128, NKT, 128], BF16, tag="vh")
                    nc.sync.dma_start(
                        out=v_h[:],
                        in_=v_sp[csl, row0:row0 + 128].rearrange("(kt p) d -> p kt d", p=128),
                    )

                    for qb in range(NQB):
                        if mask_mode == "causal":
                            kts = list(range(4 * (qb + 1)))
                        else:
                            kts = list(range(NKT))
                        exps = []
                        for kt in kts:
                            ps = ps_s.tile([128, 512], F32, tag="ps")
                            nc.tensor.matmul(
                                ps[:],
                                kr[:, kt * 128:(kt + 1) * 128],
                                qr[:, qb * 512:(qb + 1) * 512],
                                start=True, stop=True,
                            )
                            if mask_mode == "general":
                                mt = mload.tile([128, 512], F32, tag="mt")
                                nc.sync.dma_start(
                                    out=mt[:],
                                    in_=maskT_p[b, kt * 128:(kt + 1) * 128, qb * 512:(qb + 1) * 512],
                                )
                                nc.vector.tensor_add(ps[:], ps[:], mt[:])
                            e = et.tile([128, 512], BF16, tag="e")
                            nc.scalar.activation(e[:], ps[:], mybir.ActivationFunctionType.Exp, scale=INV_SQRT_D)
                            if mask_mode == "causal" and kt >= 4 * qb:
                                j = kt - 4 * qb
                                e2 = etd.tile([128, 512], BF16, tag="e2")
                                nc.gpsimd.affine_select(
                                    out=e2[:], in_=e[:],
                                    compare_op=mybir.AluOpType.is_ge, fill=0.0,
                                    base=-128 * j, pattern=[[1, 512]], channel_multiplier=-1,
                                )
                                e = e2
                            exps.append(e)

                        # pairwise bf16 tree for the k-sums
                        cur = list(exps)
                        while len(cur) > 1:
                            nxt = []
                            for i in range(0, len(cur) - 1, 2):
                                tt_ = tr.tile([128, 512], BF16, tag="tr")
                                nc.vector.tensor_add(tt_[:], cur[i][:], cur[i + 1][:])
                                nxt.append(tt_)
                            if len(cur) % 2:
                                nxt.append(cur[-1])
                            cur = nxt

                        psum_r = ps_sum.tile([128, 512], F32, tag="pssum")
                        nc.tensor.matmul(psum_r[:], ones_sb[:], cur[0][:], start=True, stop=True)
                        recip = rcp.tile([128, 512], F32, tag="rcp")
                        nc.vector.reciprocal(recip[:], psum_r[:])

                        po = ps_o.tile([128, 512], F32, tag="pso")
                        for i, kt in enumerate(kts):
                            nc.tensor.matmul(
                                po[:],
                                v_h[:, kt, :],
                                exps[i][:],
                                start=(i == 0),
                                stop=(i == len(kts) - 1),
                            )
                        nc.vector.tensor_tensor(
                            out=ao_sb[:, h, b * S + qb * 512: b * S + (qb + 1) * 512],
                            in0=po[:], in1=recip[:], op=mybir.AluOpType.mult,
                        )

            for p in reversed(phase_b):
                p.__exit__(None, None, None)

            # ---------------- phase C: o_proj ----------------
            with (
                tc.tile_pool(name="wo", bufs=1) as wo_pool,
                tc.tile_pool(name="stg", bufs=4) as stg,
                tc.tile_pool(name="pc", bufs=4, space="PSUM") as pc,
            ):
                wo_sb = wo_pool.tile([128, HPC, H], BF16, tag="wo")
                nc.sync.dma_start(out=wo_sb[:], in_=woT_p[:, :].rearrange("(ht p) o -> p ht o", p=128))

                for tt in range(T // 128):
                    for ob in range(H // 512):
                        po = pc.tile([128, 512], F32, tag="pc")
                        for ht in range(HPC):
                            nc.tensor.matmul(
                                po[:],
                                ao_sb[:, ht, tt * 128:(tt + 1) * 128],
                                wo_sb[:, ht, ob * 512:(ob + 1) * 512],
                                start=(ht == 0),
                                stop=(ht == HPC - 1),
                            )
                        st = stg.tile([128, 512], F32, tag="st")
                        nc.scalar.activation(st[:], po[:], mybir.ActivationFunctionType.Copy, scale=float(so))
                        nc.sync.dma_start(
                            out=out_p[tt * 128:(tt + 1) * 128, ob * 512:(ob + 1) * 512],
                            in_=st[:],
                        )

    _split_waits(nc)
    return nc


def _get_program(mask_mode, sq, sk, sv, so):
    key = (mask_mode, float(sq), float(sk), float(sv), float(so))
    if key not in _prog_cache:
        _prog_cache[key] = _build(mask_mode, sq, sk, sv, so)
    return _prog_cache[key]


def _detect_mask_mode(mask):
    m = np.asarray(mask)
    if not m.any():
        return "zeros"
    causal = np.triu(np.full((S, S), -1e9, dtype=np.float32), k=1)
    for b in range(m.shape[0]):
        if not np.array_equal(m[b, 0], causal):
            return "general"
    return "causal"


def kernel(hidden_states, attention_mask, wq, wk, wv, wo, sq, sk, sv, so,
           _trace=False, _trace_kwargs=None):
    x = np.asarray(hidden_states, dtype=np.float32).reshape(T, H)
    mask_mode = _detect_mask_mode(attention_mask)
    nc = _get_program(mask_mode, float(sq), float(sk), float(sv), float(so))

    xT = np.ascontiguousarray(x.T).astype(bf16)
    wqT = np.asarray(wq, dtype=np.float32).astype(bf16).T   # [H, H] bf16 (exact ternary)
    wkT = np.asarray(wk, dtype=np.float32).astype(bf16).T
    wvT = np.asarray(wv, dtype=np.float32).astype(bf16).T
    woT = np.asarray(wo, dtype=np.float32).astype(bf16).T

    in_maps = []
    for c in range(NCORES):
        sl = slice(c * OC, (c + 1) * OC)
        im = {
            "xT": xT,
            "wqT": np.ascontiguousarray(wqT[:, sl]),
            "wkT": np.ascontiguousarray(wkT[:, sl]),
            "wvT": np.ascontiguousarray(wvT[:, sl]),
            "woT": np.ascontiguousarray(woT[sl, :]),
        }
        if mask_mode == "general":
            im["maskT"] = np.ascontiguousarray(
                np.asarray(attention_mask, dtype=np.float32)[:, 0].transpose(0, 2, 1))
        in_maps.append(im)

    res = run_bass_kernel_spmd(nc, in_maps, list(range(NCORES)),
                               trace=_trace, **(_trace_kwargs or {}))
    total = np.zeros((T, H), dtype=np.float32)
    for c in range(NCORES):
        total += res.results[c]["out"]
    out = total.reshape(B, S, H)
    if _trace:
        return out, res
    return out
